# revision 7
# baseline (speedup 1.0000x reference)
"""Bahdanau-attention GRU decoder step on 8 Trainium2 NeuronCores.

Data-parallel over batch (16 batches/core); weights replicated. The big
enc_output tensor is shipped in two layouts (per-batch transposed for the
score matmul, natural for the context matmul) so every DMA is contiguous.
Matmul operands are stored in DT (float32r by default: fp32 storage,
full-rate PE); fp32 is kept wherever data is used elementwise.
"""

from contextlib import ExitStack

import numpy as np

import concourse.bacc as bacc
import concourse.bass as bass
import concourse.mybir as mybir
import concourse.tile as tile
from concourse.bass_utils import run_bass_kernel_spmd

N_CORES = 8
B, S, U, IN = 128, 1024, 1024, 64
BL = B // N_CORES  # batches per core
KC = U // 128      # contraction chunks
MC = U // 128      # dec-unit chunks
SBLK = 512         # moving-dim block over S
NSB = S // SBLK
QB = 4             # batches per softmax/context quad
G3 = 3 * U // 128  # 24 gate chunks

MODE = "f32r"      # "f32" | "f32r" | "bf16"

F32 = mybir.dt.float32
AF = mybir.ActivationFunctionType
AX = mybir.AxisListType


def _dt():
    return {
        "f32": F32,
        "f32r": mybir.dt.float32r,
        "bf16": mybir.dt.bfloat16,
    }[MODE]


def _f32view(ap):
    """View a float32r AP as plain f32 for elementwise/DMA use."""
    if ap.dtype == mybir.dt.float32r:
        return ap.bitcast(F32)
    return ap


def build_nc():
    nc = bacc.Bacc(
        "TRN2", target_bir_lowering=False, debug=False, num_devices=N_CORES
    )
    DT = _dt()

    # ---- DRAM I/O ----
    encT = nc.dram_tensor("encT", [BL, U, S], DT, kind="ExternalInput").ap()
    encN = nc.dram_tensor("encN", [BL, S, U], DT, kind="ExternalInput").ap()
    W1 = nc.dram_tensor("W1", [U, U], DT, kind="ExternalInput").ap()
    W2 = nc.dram_tensor("W2", [U, U], DT, kind="ExternalInput").ap()
    hT = nc.dram_tensor("hT", [U, BL], DT, kind="ExternalInput").ap()
    hTf = nc.dram_tensor("hTf", [U, BL], F32, kind="ExternalInput").ap()
    Vr = nc.dram_tensor("Vr", [128, MC], DT, kind="ExternalInput").ap()
    b12r = nc.dram_tensor("b12r", [128, MC], F32, kind="ExternalInput").ap()
    eye16 = nc.dram_tensor("eye16", [16, 16], F32, kind="ExternalInput").ap()
    x2T = nc.dram_tensor("x2T", [IN, BL], DT, kind="ExternalInput").ap()
    Wk = nc.dram_tensor("Wk", [U + IN, 3 * U], DT, kind="ExternalInput").ap()
    Wr = nc.dram_tensor("Wr", [U, 3 * U], DT, kind="ExternalInput").ap()
    bgr = nc.dram_tensor("bgr", [128, G3], F32, kind="ExternalInput").ap()
    Wfc = nc.dram_tensor("Wfc", [U, IN], DT, kind="ExternalInput").ap()
    bfcr = nc.dram_tensor("bfcr", [IN, 1], F32, kind="ExternalInput").ap()

    attnW = nc.dram_tensor("attnW", [BL, S], F32, kind="ExternalOutput").ap()
    stateT = nc.dram_tensor("stateT", [U, BL], F32, kind="ExternalOutput").ap()
    outT = nc.dram_tensor("outT", [IN, BL], F32, kind="ExternalOutput").ap()

    with tile.TileContext(nc) as tc, ExitStack() as ctx:
        cpool = ctx.enter_context(tc.tile_pool(name="const", bufs=1))
        w1p = ctx.enter_context(tc.tile_pool(name="w1", bufs=1))

        # ---- constants / small inputs ----
        eye_sb = cpool.tile([16, 16], F32, tag="eye")
        nc.sync.dma_start(eye_sb[:], eye16)
        vr_sb = cpool.tile([128, MC], DT, tag="vr")
        nc.sync.dma_start(vr_sb[:], Vr)
        b12_sb = cpool.tile([128, MC], F32, tag="b12")
        nc.sync.dma_start(b12_sb[:], b12r)
        hT_sb = cpool.tile([128, KC, BL], DT, tag="hT")
        nc.sync.dma_start(hT_sb[:], hT.rearrange("(k p) b -> p k b", p=128))
        hTf_sb = cpool.tile([128, KC, BL], F32, tag="hTf")
        nc.sync.dma_start(hTf_sb[:], hTf.rearrange("(k p) b -> p k b", p=128))
        qT_sb = cpool.tile([128, MC, BL], F32, tag="qT")
        attnT_sb = cpool.tile([128, KC, BL], DT, tag="attnT")
        ctx_sb = cpool.tile([BL, U], F32, tag="ctxrow")

        w1_sb = []
        for k in range(KC):
            t = w1p.tile([128, U], DT, tag=f"w1_{k}")
            nc.sync.dma_start(t[:], W1[k * 128:(k + 1) * 128, :])
            w1_sb.append(t)

        # ---- phase 0: qT = (hidden @ W2 + b1 + b2)^T ----
        with tc.tile_pool(name="w2s", bufs=2) as w2p, \
             tc.tile_pool(name="ph0ps", bufs=8, space="PSUM") as pps:
            qps = [pps.tile([128, BL], F32, tag="qps", name=f"qps{m}")
                   for m in range(MC)]
            for k in range(KC):
                w2t = w2p.tile([128, U], DT, tag="w2t")
                nc.sync.dma_start(w2t[:], W2[k * 128:(k + 1) * 128, :])
                for m in range(MC):
                    nc.tensor.matmul(
                        qps[m][:],
                        lhsT=w2t[:, m * 128:(m + 1) * 128],
                        rhs=hT_sb[:, k, :],
                        start=(k == 0),
                        stop=(k == KC - 1),
                    )
            for m in range(MC):
                nc.scalar.activation(
                    qT_sb[:, m, :], qps[m][:], AF.Identity,
                    bias=b12_sb[:, m:m + 1],
                )

        # ---- phases 1-3 per quad: scores -> softmax -> context ----
        with tc.tile_pool(name="encT", bufs=2) as etp, \
             tc.tile_pool(name="tf", bufs=16) as tfp, \
             tc.tile_pool(name="encN", bufs=3) as enp, \
             tc.tile_pool(name="sm", bufs=4) as smp, \
             tc.tile_pool(name="featps", bufs=3, space="PSUM") as fps, \
             tc.tile_pool(name="scoreps", bufs=1, space="PSUM") as sps, \
             tc.tile_pool(name="ctxps", bufs=2, space="PSUM") as cps, \
             tc.tile_pool(name="tpps", bufs=2, space="PSUM") as tps:

            for q in range(BL // QB):
                sq = smp.tile([QB, S], F32, tag="sq")
                for bq in range(QB):
                    b = q * QB + bq
                    for sb in range(NSB):
                        et = etp.tile([128, KC, SBLK], DT, tag="et")
                        nc.sync.dma_start(
                            et[:],
                            encT[b].rearrange("(k p) s -> p k s", p=128)
                            [:, :, sb * SBLK:(sb + 1) * SBLK],
                        )
                        tfs = []
                        for m in range(MC):
                            fp = fps.tile([128, SBLK], F32, tag="fp")
                            for k in range(KC):
                                nc.tensor.matmul(
                                    fp[:],
                                    lhsT=w1_sb[k][:, m * 128:(m + 1) * 128],
                                    rhs=et[:, k, :],
                                    start=(k == 0),
                                    stop=(k == KC - 1),
                                )
                            tf = tfp.tile([128, SBLK], DT, tag="tf")
                            nc.scalar.activation(
                                tf[:], fp[:], AF.Tanh, bias=qT_sb[:, m, b:b + 1]
                            )
                            tfs.append(tf)
                        sp = sps.tile([1, SBLK], F32, tag="sp")
                        for m in range(MC):
                            nc.tensor.matmul(
                                sp[:],
                                lhsT=vr_sb[:, m:m + 1],
                                rhs=tfs[m][:],
                                start=(m == 0),
                                stop=(m == MC - 1),
                            )
                        srow = smp.tile([1, SBLK], F32, tag="srow")
                        nc.scalar.activation(srow[:], sp[:], AF.Identity)
                        nc.sync.dma_start(
                            sq[bq:bq + 1, sb * SBLK:(sb + 1) * SBLK], srow[:]
                        )

                # softmax over S for the quad
                nm = smp.tile([QB, 1], F32, tag="nm")
                nc.vector.reduce_max(nm[:], sq[:], axis=AX.X, negate=True)
                aq = smp.tile([QB, S], F32, tag="aq")
                se = smp.tile([QB, 1], F32, tag="se")
                nc.scalar.activation(
                    aq[:], sq[:], AF.Exp, bias=nm[:], accum_out=se[:]
                )
                rv = smp.tile([QB, 1], F32, tag="rv")
                nc.vector.reciprocal(rv[:], se[:])
                aw = smp.tile([QB, S], F32, tag="aw")
                nc.vector.tensor_scalar_mul(aw[:], aq[:], rv[:])
                nc.sync.dma_start(attnW[q * QB:(q + 1) * QB, :], aw[:])
                for c in range(KC):
                    tp = tps.tile([128, QB], F32, tag="tp")
                    nc.tensor.transpose(
                        tp[:], aw[:, c * 128:(c + 1) * 128], eye_sb[:QB, :QB]
                    )
                    nc.scalar.activation(
                        attnT_sb[:, c, q * QB:(q + 1) * QB], tp[:], AF.Identity
                    )

                # context for the quad
                for bq in range(QB):
                    b = q * QB + bq
                    c0 = cps.tile([1, 512], F32, tag="ctx", name="c0")
                    c1 = cps.tile([1, 512], F32, tag="ctx", name="c1")
                    for sc in range(KC):
                        en = enp.tile([128, U], DT, tag="en")
                        nc.sync.dma_start(
                            en[:], encN[b, sc * 128:(sc + 1) * 128, :]
                        )
                        nc.tensor.matmul(
                            c0[:], lhsT=attnT_sb[:, sc, b:b + 1],
                            rhs=en[:, 0:512],
                            start=(sc == 0), stop=(sc == KC - 1),
                        )
                        nc.tensor.matmul(
                            c1[:], lhsT=attnT_sb[:, sc, b:b + 1],
                            rhs=en[:, 512:1024],
                            start=(sc == 0), stop=(sc == KC - 1),
                        )
                    crow = smp.tile([1, U], F32, tag="crow")
                    nc.scalar.activation(crow[:, 0:512], c0[:], AF.Identity)
                    nc.scalar.activation(crow[:, 512:1024], c1[:], AF.Identity)
                    nc.sync.dma_start(ctx_sb[b:b + 1, :], crow[:])

        # ---- phase 4: GRU cell + fc, all in [unit, batch] layout ----
        with tc.tile_pool(name="gconst", bufs=1) as gcp, \
             tc.tile_pool(name="wks", bufs=3) as wkp, \
             tc.tile_pool(name="wrs", bufs=3) as wrp, \
             tc.tile_pool(name="xgps", bufs=4, space="PSUM") as xps, \
             tc.tile_pool(name="tp2ps", bufs=2, space="PSUM") as tp2, \
             tc.tile_pool(name="fcps", bufs=1, space="PSUM") as fcp:

            # transpose context rows -> ctxT chunks (cast to DT)
            ctxT_sb = gcp.tile([128, KC, BL], DT, tag="ctxT")
            for c in range(KC):
                tp = tp2.tile([128, BL], F32, tag="tpc")
                nc.tensor.transpose(
                    tp[:], ctx_sb[:, c * 128:(c + 1) * 128], eye_sb[:]
                )
                nc.scalar.activation(ctxT_sb[:, c, :], tp[:], AF.Identity)

            x2T_sb = gcp.tile([IN, BL], DT, tag="x2T")
            nc.sync.dma_start(x2T_sb[:], x2T)
            bg_sb = gcp.tile([128, G3], F32, tag="bg")
            nc.sync.dma_start(bg_sb[:], bgr)

            zT = gcp.tile([128, KC, BL], F32, tag="zT")
            rT = gcp.tile([128, KC, BL], F32, tag="rT")
            hhT = gcp.tile([128, KC, BL], F32, tag="hhT")
            rhT = gcp.tile([128, KC, BL], DT, tag="rhT")

            def xg_group(g, hid_rhs):
                """Accumulate 4 gate chunks j=4g..4g+3; returns psum tiles."""
                cols = slice(4 * g * 128, (4 * g + 4) * 128)
                xg = [xps.tile([128, BL], F32, tag="xg", name=f"xg{g}_{jj}")
                      for jj in range(4)]
                for k in range(KC + 1):
                    rows = 128 if k < KC else IN
                    wkt = wkp.tile([rows, 512], DT, tag="wkt")
                    nc.sync.dma_start(
                        wkt[:rows, :], Wk[k * 128:k * 128 + rows, cols]
                    )
                    rhs = ctxT_sb[:, k, :] if k < KC else x2T_sb[:]
                    for jj in range(4):
                        nc.tensor.matmul(
                            xg[jj][:],
                            lhsT=wkt[:rows, jj * 128:(jj + 1) * 128],
                            rhs=rhs,
                            start=(k == 0), stop=False,
                        )
                for k in range(KC):
                    wrt = wrp.tile([128, 512], DT, tag="wrt")
                    nc.sync.dma_start(wrt[:], Wr[k * 128:(k + 1) * 128, cols])
                    for jj in range(4):
                        nc.tensor.matmul(
                            xg[jj][:],
                            lhsT=wrt[:, jj * 128:(jj + 1) * 128],
                            rhs=hid_rhs(k),
                            start=False, stop=(k == KC - 1),
                        )
                return xg

            # z and r gates (groups 0..3)
            for g in range(4):
                xg = xg_group(g, lambda k: hT_sb[:, k, :])
                dst = zT if g < 2 else rT
                for jj in range(4):
                    j = 4 * g + jj
                    nc.scalar.activation(
                        dst[:, j % KC, :], xg[jj][:], AF.Sigmoid,
                        bias=bg_sb[:, j:j + 1],
                    )
            # r * hidden (DT output feeds the hh matmul)
            for c in range(KC):
                nc.vector.tensor_mul(
                    rhT[:, c, :], rT[:, c, :], hTf_sb[:, c, :]
                )
            # hh pre-activation (groups 4..5)
            for g in range(4, 6):
                xg = xg_group(g, lambda k: rhT[:, k, :])
                for jj in range(4):
                    j = 4 * g + jj
                    nc.scalar.activation(
                        hhT[:, j % KC, :], xg[jj][:], AF.Tanh,
                        bias=bg_sb[:, j:j + 1],
                    )
            # state = hh + z * (h - hh); DT copy for the fc matmul
            sT = gcp.tile([128, KC, BL], F32, tag="sT")
            dT = gcp.tile([128, KC, BL], F32, tag="dT")
            sTd = gcp.tile([128, KC, BL], DT, tag="sTd")
            for c in range(KC):
                nc.vector.tensor_sub(dT[:, c, :], hTf_sb[:, c, :], hhT[:, c, :])
                nc.vector.tensor_mul(dT[:, c, :], zT[:, c, :], dT[:, c, :])
                nc.vector.tensor_add(sT[:, c, :], hhT[:, c, :], dT[:, c, :])
                nc.scalar.activation(sTd[:, c, :], sT[:, c, :], AF.Identity)
                nc.sync.dma_start(stateT[c * 128:(c + 1) * 128, :], sT[:, c, :])

            # fc: out^T = Wfc^T @ state^T + bfc
            wfc_sb = gcp.tile([128, KC, IN], DT, tag="wfc")
            nc.sync.dma_start(wfc_sb[:], Wfc.rearrange("(k p) o -> p k o", p=128))
            bfc_sb = gcp.tile([IN, 1], F32, tag="bfc")
            nc.sync.dma_start(bfc_sb[:], bfcr)
            fc = fcp.tile([IN, BL], F32, tag="fc")
            for k in range(KC):
                nc.tensor.matmul(
                    fc[:],
                    lhsT=wfc_sb[:, k, :],
                    rhs=sTd[:, k, :],
                    start=(k == 0), stop=(k == KC - 1),
                )
            ot = gcp.tile([IN, BL], F32, tag="ot")
            nc.scalar.activation(ot[:], fc[:], AF.Identity, bias=bfc_sb[:])
            nc.sync.dma_start(outT, ot[:])

    nc.compile()
    return nc


_NC_CACHE = {}
LAST_RESULT = None


def _get_nc():
    if MODE not in _NC_CACHE:
        _NC_CACHE[MODE] = build_nc()
    return _NC_CACHE[MODE]


def kernel(x, hidden, enc_output, W1, b1, W2, b2, V, bV, Wk, Wr, bg, Wfc, bfc):
    global LAST_RESULT
    x = np.asarray(x, dtype=np.float32)
    hidden = np.asarray(hidden, dtype=np.float32)
    enc_output = np.asarray(enc_output, dtype=np.float32)
    W1 = np.asarray(W1, dtype=np.float32)
    b1 = np.asarray(b1, dtype=np.float32)
    W2 = np.asarray(W2, dtype=np.float32)
    b2 = np.asarray(b2, dtype=np.float32)
    V = np.asarray(V, dtype=np.float32)
    Wk = np.asarray(Wk, dtype=np.float32)
    Wr = np.asarray(Wr, dtype=np.float32)
    bg = np.asarray(bg, dtype=np.float32)
    Wfc = np.asarray(Wfc, dtype=np.float32)
    bfc = np.asarray(bfc, dtype=np.float32)

    if MODE == "bf16":
        import ml_dtypes
        dtm_np = ml_dtypes.bfloat16
    else:
        dtm_np = np.float32

    # replicated weights / layouts
    W1m = np.ascontiguousarray(W1).astype(dtm_np)
    W2m = np.ascontiguousarray(W2).astype(dtm_np)
    Vr = np.ascontiguousarray(V[:, 0].reshape(MC, 128).T).astype(dtm_np)
    b12r = np.ascontiguousarray((b1 + b2).reshape(MC, 128).T)
    eye = np.eye(16, dtype=np.float32)
    Wkm = np.ascontiguousarray(Wk).astype(dtm_np)
    Wrm = np.ascontiguousarray(Wr).astype(dtm_np)
    bgr = np.ascontiguousarray(bg.reshape(G3, 128).T)
    bfcr = bfc.reshape(IN, 1)
    Wfcm = np.ascontiguousarray(Wfc).astype(dtm_np)

    in_maps = []
    for c in range(N_CORES):
        bs = slice(c * BL, (c + 1) * BL)
        enc_c = enc_output[bs]
        hT_c = np.ascontiguousarray(hidden[bs].T)
        m = {
            "encT": np.ascontiguousarray(enc_c.transpose(0, 2, 1)).astype(dtm_np),
            "encN": np.ascontiguousarray(enc_c).astype(dtm_np),
            "W1": W1m,
            "W2": W2m,
            "hT": hT_c.astype(dtm_np),
            "hTf": hT_c,
            "Vr": Vr,
            "b12r": b12r,
            "eye16": eye,
            "x2T": np.ascontiguousarray(x[bs, 0, :].T).astype(dtm_np),
            "Wk": Wkm,
            "Wr": Wrm,
            "bgr": bgr,
            "Wfc": Wfcm,
            "bfcr": bfcr,
        }
        in_maps.append(m)

    nc = _get_nc()
    res = run_bass_kernel_spmd(nc, in_maps, list(range(N_CORES)))
    LAST_RESULT = res

    out = np.empty((B, IN), np.float32)
    state = np.empty((B, U), np.float32)
    attn = np.empty((B, S, 1), np.float32)
    for c in range(N_CORES):
        bs = slice(c * BL, (c + 1) * BL)
        r = res.results[c]
        out[bs] = r["outT"].T
        state[bs] = r["stateT"].T
        attn[bs] = r["attnW"][:, :, None]
    return out, state, attn
